# revision 2
# baseline (speedup 1.0000x reference)
"""Trainium2 Bass kernel v4 for nn_DendroMatrixNN (B=4096, F=256, L0=128, L1=16,
E=2048, N=2048).

Per-node weight tables built by one big matmul (path_sel^T @ delta^T), then
per-sample gather + dot products.  Key structure:

* Mixed-precision table matmul: ACH bf16 chunks + (16-ACH)/2 fp8e4 DoubleRow
  pairs (K=256/instr, 2x PE throughput) over the E=2048 contraction.  Inputs
  pre-scaled by 64 (fp8 normal range); eviction divides by 64.  A rank-1
  density correction (-0.5*colsum(quant error) folded into the root row)
  cancels half the fp8 error variance for free.
* Bank-outer loop, delta streamed from HBM exactly once.  Table banks are
  grouped into wave tensors {b0b1, b2b3, b4b5, b6, b7} so stage-B overlaps
  the PE and the serial tail is a single 512-column bank.
* Stage-B: TG sample tiles per dma_gather (one SWDGE instruction fetches
  TG*128 table rows), one DVE bf16 multiply per (tile, wave), per-l sums on
  the Activation engine's accumulator, relu + layer-2 per tile in the final
  wave.

Cores shard L0 (16 l-values per core); host sums the 8 partial outputs.
"""

import os as _os

import ml_dtypes
import numpy as np

BF16 = ml_dtypes.bfloat16
F8 = ml_dtypes.float8_e4m3fn

# Problem dims (hardcoded per spec nn_DendroMatrixNN_19301583028815)
B, F, L0, L1 = 4096, 256, 128, 16
E, N = 2048, 2048
NCORES = 8
SCALE = 64.0

ACH = int(_os.environ.get("KERNEL_ACH", "6"))      # bf16 chunks of 16
assert ACH % 2 == 0 and 0 <= ACH <= 16
NPR = (16 - ACH) // 2                              # fp8 DoubleRow pairs
TG = int(_os.environ.get("KERNEL_TG", "4"))        # sample tiles per dma_gather
DOTS = _os.environ.get("KERNEL_DOTS", "act")       # act | dve stage-B sums
GMODE = _os.environ.get("KERNEL_GATHER", "indirect")  # indirect | gather
SINGLE_PACKET = _os.environ.get("KERNEL_SP", "1") == "1"

# wave -> list of main banks (bank = 512 table cols = 2 l-values)
WAVES = [[0, 1], [2, 3], [4, 5], [6], [7]]


def make_cfg(b=B, f=F, l0=L0, l1=L1, e=E, n=N, ncores=NCORES):
    lc = l0 // ncores          # l-values per core
    m1 = lc * f                # main table columns per core
    m2 = lc * l1               # tail table columns per core
    assert l0 % ncores == 0 and b % 128 == 0 and e % 128 == 0
    assert m1 % 512 == 0 and m2 <= 512
    return dict(
        b=b, f=f, l0=l0, l1=l1, e=e, n=n, ncores=ncores,
        lc=lc, m1=m1, m2=m2, m=m1 + m2,
        ec=e // 128, tc=b // 128, nb1=m1 // 512,
    )


def build_program(cfg, nkc):
    from contextlib import ExitStack

    import concourse.bass as bass
    import concourse.tile as tile
    from concourse import bacc, mybir

    f32 = mybir.dt.float32
    bf = mybir.dt.bfloat16
    f8 = mybir.dt.float8e4
    i16 = mybir.dt.int16
    Alu = mybir.AluOpType
    Act = mybir.ActivationFunctionType
    Axis = mybir.AxisListType
    DR = mybir.MatmulPerfMode.DoubleRow

    f, l1, lc = cfg["f"], cfg["l1"], cfg["lc"]
    m1, m2 = cfg["m1"], cfg["m2"]
    ec, nb1, tc = cfg["ec"], cfg["nb1"], cfg["tc"]
    n_eff = nkc * 128
    ngr = tc // TG                       # gather groups per wave
    bank_wave = {}                       # bank -> (wave, col offset)
    for w, bs in enumerate(WAVES):
        for i, bi in enumerate(bs):
            bank_wave[bi] = (w, i * 512)

    nc = bacc.Bacc("TRN2", target_bir_lowering=False, debug=False)

    # ---- I/O ---------------------------------------------------------------
    path_bf = nc.dram_tensor("path_bf", [128, nkc, ec, 128], bf,
                             kind="ExternalInput")
    path_f8 = nc.dram_tensor("path_f8", [128, nkc, max(NPR, 1), 2, 128], f8,
                             kind="ExternalInput")
    delta_bf = nc.dram_tensor("delta_bf", [128, nb1, max(ACH, 1), 512], bf,
                              kind="ExternalInput")
    delta_f8 = nc.dram_tensor("delta_f8", [128, nb1, max(NPR, 1), 2, 512], f8,
                              kind="ExternalInput")
    delta_tl = nc.dram_tensor("delta_tl", [128, ec, m2], bf,
                              kind="ExternalInput")
    x_t = nc.dram_tensor("x_t", [128, tc, f], bf, kind="ExternalInput")
    # idx16[j, s] (j<16, replicated to 128) = table row of sample slot s*16+j
    idx16_t = nc.dram_tensor("idx16_t", [128, tc * 8], i16,
                             kind="ExternalInput")
    idx_t = nc.dram_tensor("idx_t", [128, tc], mybir.dt.int32,
                           kind="ExternalInput")
    root_row = nc.dram_tensor("root_row", [1, m1 + m2], bf,
                              kind="ExternalInput")
    outp = nc.dram_tensor("outp", [128, tc * l1], f32, kind="ExternalOutput")
    t_wave = [nc.dram_tensor(f"t_wave{w}", [n_eff, 512 * len(bs)], bf)
              for w, bs in enumerate(WAVES)]
    t_tail = nc.dram_tensor("t_tail", [n_eff, m2], bf)

    with tile.TileContext(nc) as tc_, ExitStack() as ctx:
        from concourse import library_config
        nc.gpsimd.load_library(library_config.mlp)
        pconst = ctx.enter_context(tc_.tile_pool(name="const", bufs=1))
        pdelta = ctx.enter_context(tc_.tile_pool(name="delta", bufs=2))
        pevict = ctx.enter_context(tc_.tile_pool(name="evict", bufs=6))
        ppsum = ctx.enter_context(tc_.tile_pool(name="psum", bufs=8,
                                                space="PSUM"))
        pgather = ctx.enter_context(tc_.tile_pool(name="gather", bufs=2))
        pprod = ctx.enter_context(tc_.tile_pool(name="prod", bufs=3))
        psmall = ctx.enter_context(tc_.tile_pool(name="small", bufs=3))

        # ---- root rows broadcast to all partitions via K=1 ones-matmul ------
        root_sb = pconst.tile([1, m1 + m2], bf, tag="root")
        nc.sync.dma_start(out=root_sb[:], in_=root_row[:])
        ones_sb = pconst.tile([1, 128], bf, tag="ones")
        nc.vector.memset(ones_sb[:], 1.0)
        rrep = pconst.tile([128, m1 + m2], bf, tag="rootrep")
        banks = [(i * 512, 512) for i in range(nb1)] + [(m1, m2)]
        for bi, (col0, bw) in enumerate(banks):
            psr = ppsum.tile([128, bw], f32, tag="psum_mm", name=f"psr{bi}")
            nc.tensor.matmul(psr[:], lhsT=ones_sb[:],
                             rhs=root_sb[:, col0:col0 + bw], start=True,
                             stop=True)
            nc.vector.tensor_copy(rrep[:, col0:col0 + bw], psr[:])

        pb_nk = [pconst.tile([128, ec, 128], bf, tag=f"pb{nk}",
                             name=f"pb{nk}") for nk in range(nkc)]
        p8_nk = [pconst.tile([128, NPR, 2, 128], f8, tag=f"p8{nk}",
                             name=f"p8{nk}") for nk in range(nkc)] \
            if NPR else None
        x_sb = pconst.tile([128, tc, f], bf, tag="x")
        idx_sb = pconst.tile([128, tc * 8], i16, tag="idx")
        idx32_sb = pconst.tile([128, tc], mybir.dt.int32, tag="idx32")
        hpre_t = [pconst.tile([128, lc], f32, tag=f"hpre{t}",
                              name=f"hpre{t}") for t in range(tc)]
        g2_sb = pconst.tile([128, tc, m2], bf, tag="g2")
        out_sb = pconst.tile([128, tc * l1], f32, tag="outsb")

        def load_path(nk):
            nc.gpsimd.dma_start(out=pb_nk[nk][:], in_=path_bf[:, nk])
            if NPR:
                nc.gpsimd.dma_start(out=p8_nk[nk][:], in_=path_f8[:, nk])

        def load_delta(bi):
            if bi == nb1:
                dtl = pdelta.tile([128, ec, m2], bf, tag="dtl")
                nc.gpsimd.dma_start(out=dtl[:], in_=delta_tl[:])
                return (dtl,)
            dbf = df8 = None
            if ACH:
                dbf = pdelta.tile([128, ACH, 512], bf, tag="dbf",
                                  name=f"dbf{bi}")
                nc.gpsimd.dma_start(out=dbf[:], in_=delta_bf[:, bi])
            if NPR:
                df8 = pdelta.tile([128, NPR, 2, 512], f8, tag="df8",
                                  name=f"df8{bi}")
                nc.gpsimd.dma_start(out=df8[:], in_=delta_f8[:, bi])
            return (dbf, df8)

        def build_bank(bi, dt):
            """PE: build table bank bi into its wave tensor."""
            col0, bw = banks[bi]
            tail = bi == nb1
            for nk in range(nkc):
                ps = ppsum.tile([128, bw], f32, tag="psum_mm",
                                name=f"ps{bi}_{nk}")
                if tail:
                    (dtl,) = dt
                    for c in range(ec):
                        nc.tensor.matmul(
                            ps[:], lhsT=pb_nk[nk][:, c, :], rhs=dtl[:, c, :],
                            start=(c == 0), stop=(c == ec - 1),
                        )
                else:
                    dbf, df8 = dt
                    for c in range(ACH):
                        nc.tensor.matmul(
                            ps[:], lhsT=pb_nk[nk][:, c, :], rhs=dbf[:, c, :],
                            start=(c == 0), stop=(NPR == 0 and c == ACH - 1),
                        )
                    for pr in range(NPR):
                        nc.tensor.matmul(
                            ps[:], lhsT=p8_nk[nk][:, pr, :, :],
                            rhs=df8[:, pr, :, :],
                            start=(ACH == 0 and pr == 0),
                            stop=(pr == NPR - 1), perf_mode=DR,
                        )
                ev = pevict.tile([128, bw], bf, tag="evict",
                                 name=f"ev{bi}_{nk}")
                with tc_.high_priority():
                    nc.vector.scalar_tensor_tensor(
                        out=ev[:], in0=ps[:], scalar=1.0 / SCALE,
                        in1=rrep[:, col0:col0 + bw],
                        op0=Alu.mult, op1=Alu.add,
                    )
                if tail:
                    dst = t_tail[nk * 128:(nk + 1) * 128, :]
                else:
                    w, wcol = bank_wave[bi]
                    dst = t_wave[w][nk * 128:(nk + 1) * 128,
                                    wcol:wcol + 512]
                nc.sync.dma_start(out=dst, in_=ev[:])

        def gather_g2():
            if GMODE == "gather":
                nc.gpsimd.dma_gather(
                    g2_sb[:], t_tail[:, :], idx_sb[:, :],
                    tc * 128, tc * 128, m2, single_packet=SINGLE_PACKET,
                )
            else:
                for t in range(tc):
                    nc.gpsimd.indirect_dma_start(
                        out=g2_sb[:, t, :], out_offset=None, in_=t_tail[:, :],
                        in_offset=bass.IndirectOffsetOnAxis(
                            ap=idx32_sb[:, t:t + 1], axis=0),
                    )

        def stage_b_wave(w, final=False):
            wb = WAVES[w]
            width = 512 * len(wb)
            nseg = 2 * len(wb)           # l-values in this wave
            l0w = 2 * wb[0]
            for g in range(ngr):
                G = pgather.tile([128, TG, width], bf, tag="G",
                                 name=f"G{w}_{g}")
                if GMODE == "gather":
                    nc.gpsimd.dma_gather(
                        G[:], t_wave[w][:, :],
                        idx_sb[:, g * TG * 8:(g + 1) * TG * 8],
                        TG * 128, TG * 128, width, single_packet=SINGLE_PACKET,
                    )
                else:
                    for i in range(TG):
                        t = g * TG + i
                        nc.gpsimd.indirect_dma_start(
                            out=G[:, i, :], out_offset=None,
                            in_=t_wave[w][:, :],
                            in_offset=bass.IndirectOffsetOnAxis(
                                ap=idx32_sb[:, t:t + 1], axis=0),
                        )
                for i in range(TG):
                    t = g * TG + i
                    if DOTS == "act":
                        prodb = pprod.tile([128, nseg, f], bf, tag="prodb",
                                           name=f"pr{w}_{t}")
                        nc.vector.tensor_tensor(
                            out=prodb[:],
                            in0=G[:, i, :].rearrange("p (s e) -> p s e", e=f),
                            in1=x_sb[:, t, None, :].to_broadcast(
                                [128, nseg, f]),
                            op=Alu.mult,
                        )
                        dummy = psmall.tile([128, f], bf, tag="dummy",
                                            name=f"dm{w}_{t}")
                        for s in range(nseg):
                            nc.scalar.activation(
                                out=dummy[:], in_=prodb[:, s, :],
                                func=Act.Identity,
                                accum_out=hpre_t[t][:, l0w + s:l0w + s + 1],
                            )
                    else:
                        dummy = psmall.tile([128, f], bf, tag="dummy",
                                            name=f"dm{w}_{t}")
                        for s in range(nseg):
                            nc.vector.scalar_tensor_tensor(
                                out=dummy[:],
                                in0=G[:, i, s * f:(s + 1) * f], scalar=1.0,
                                in1=x_sb[:, t, :], op0=Alu.bypass,
                                op1=Alu.mult,
                                accum_out=hpre_t[t][:, l0w + s:l0w + s + 1],
                            )
                    if final:
                        finish_tile(t)

        def finish_tile(t):
            h = psmall.tile([128, lc], f32, tag="h", name=f"h{t}")
            nc.scalar.activation(out=h[:], in_=hpre_t[t][:], func=Act.Relu)
            prod = psmall.tile([128, l1, lc], f32, tag="prod",
                               name=f"prod{t}")
            nc.vector.tensor_tensor(
                out=prod[:],
                in0=g2_sb[:, t, :].rearrange("p (m_ l_) -> p m_ l_", l_=lc),
                in1=h[:, None, :].to_broadcast([128, l1, lc]),
                op=Alu.mult,
            )
            nc.vector.tensor_reduce(
                out=out_sb[:, t * l1:(t + 1) * l1], in_=prod[:],
                axis=Axis.X, op=Alu.add,
            )

        # Load order = PE need order.
        load_path(0)
        dt0 = load_delta(0)
        for nk in range(1, nkc):
            load_path(nk)
        dt_tail = load_delta(nb1)
        nc.gpsimd.dma_start(out=x_sb[:], in_=x_t[:])
        nc.gpsimd.dma_start(out=idx_sb[:], in_=idx16_t[:])
        nc.gpsimd.dma_start(out=idx32_sb[:], in_=idx_t[:])

        # PE order: bank0, tail, banks 1..7.  Wave stage-B runs as soon as its
        # banks' tables are written, overlapping later banks' PE work.
        build_bank(0, dt0)
        build_bank(nb1, dt_tail)
        gather_g2()
        done = {0}
        emitted = set()
        for bi in range(1, nb1):
            dt = load_delta(bi)
            build_bank(bi, dt)
            done.add(bi)
            for w, bs in enumerate(WAVES):
                if w not in emitted and all(b in done for b in bs):
                    stage_b_wave(w, final=(w == len(WAVES) - 1))
                    emitted.add(w)
        nc.sync.dma_start(out=outp[:], in_=out_sb[:])

    nc.compile()
    return nc


def host_prep(cfg, x, node_idx, path_mat, root_lin1, root_lin2, delta_mat1,
              delta_mat2):
    """Dedup nodes, sort samples by node rank, quantize + relayout per core."""
    f, l1, e = cfg["f"], cfg["l1"], cfg["e"]
    lc, m1, m2 = cfg["lc"], cfg["m1"], cfg["m2"]
    ec, nb1, tc = cfg["ec"], cfg["nb1"], cfg["tc"]

    x = np.asarray(x, np.float32)
    node_idx = np.asarray(node_idx, np.int32)
    path_mat = np.asarray(path_mat, np.float32)
    root_lin1 = np.asarray(root_lin1, np.float32)
    root_lin2 = np.asarray(root_lin2, np.float32)
    delta_mat1 = np.asarray(delta_mat1, np.float32)
    delta_mat2 = np.asarray(delta_mat2, np.float32)

    uniq, inv = np.unique(node_idx, return_inverse=True)
    k = len(uniq)
    nkc = -(-k // 128)
    n_eff = nkc * 128
    path_sel = np.zeros((e, n_eff), np.float32)
    path_sel[:, :k] = path_mat[:, uniq]

    order = np.argsort(inv, kind="stable")     # samples sorted by node rank
    local_idx = inv[order].astype(np.int32)

    # x_t[p, t, :] = x of sample slot t*128+p
    x_t = np.ascontiguousarray(
        x[order].reshape(tc, 128, f).transpose(1, 0, 2), BF16)
    # dma_gather index layout: index of slot s lives at [s % 16, s // 16],
    # replicated across the 8 16-partition groups.
    idx16 = np.ascontiguousarray(
        local_idx.reshape(tc * 8, 16).T).astype(np.int16)   # [16, tc*8]
    idx16_t = np.tile(idx16, (8, 1))                        # [128, tc*8]
    idx_t = np.ascontiguousarray(local_idx.reshape(tc, 128).T)

    p4 = path_sel.reshape(ec, 128, nkc, 128)
    path_bf = np.ascontiguousarray(p4.transpose(1, 2, 0, 3), BF16)
    if NPR:
        path_f8 = np.ascontiguousarray(
            p4[ACH:].reshape(NPR, 2, 128, nkc, 128).transpose(2, 3, 0, 1, 4),
            F8)
    else:
        path_f8 = np.zeros((128, nkc, 1, 2, 128), F8)

    in_maps = []
    for c in range(cfg["ncores"]):
        lsl = slice(c * lc, (c + 1) * lc)
        d1t = (delta_mat1[lsl].reshape(lc * f, e).T * SCALE).astype(np.float32)
        d2t = (delta_mat2[:, lsl, :].transpose(2, 0, 1).reshape(e, lc * l1)
               * SCALE).astype(np.float32)                      # cols (m, l)

        d1q = np.empty_like(d1t)
        d1q[:ACH * 128] = d1t[:ACH * 128].astype(BF16)
        if NPR:
            d1q[ACH * 128:] = d1t[ACH * 128:].astype(F8)
        colsum_eps1 = (d1q - d1t).sum(0) / SCALE                 # [m1]
        d2q = d2t.astype(BF16).astype(np.float32)
        colsum_eps2 = (d2q - d2t).sum(0) / SCALE                 # [m2]

        dm = d1q.reshape(ec, 128, m1)
        delta_bf = np.ascontiguousarray(
            dm[:ACH].reshape(ACH, 128, nb1, 512).transpose(1, 2, 0, 3),
            BF16) if ACH else np.zeros((128, nb1, 1, 512), BF16)
        if NPR:
            delta_f8 = np.ascontiguousarray(
                dm[ACH:].reshape(NPR, 2, 128, nb1, 512)
                .transpose(2, 3, 0, 1, 4), F8)
        else:
            delta_f8 = np.zeros((128, nb1, 1, 2, 512), F8)
        delta_tl = np.ascontiguousarray(
            d2q.reshape(ec, 128, m2).transpose(1, 0, 2), BF16)

        root_row = np.concatenate([
            root_lin1[:, lsl].T.reshape(-1) - 0.5 * colsum_eps1,
            root_lin2[lsl, :].T.reshape(-1) - 0.5 * colsum_eps2,
        ]).astype(BF16)[None, :]
        in_maps.append({
            "path_bf": path_bf, "path_f8": path_f8,
            "delta_bf": delta_bf, "delta_f8": delta_f8, "delta_tl": delta_tl,
            "x_t": x_t, "idx16_t": idx16_t, "idx_t": idx_t,
            "root_row": np.ascontiguousarray(root_row),
        })
    return in_maps, nkc, order


def host_finish(cfg, per_core_outs, order):
    b, l1, tc = cfg["b"], cfg["l1"], cfg["tc"]
    tot = np.zeros((128, tc * l1), np.float32)
    for o in per_core_outs:
        tot += o.reshape(128, tc * l1)
    slots = tot.reshape(128, tc, l1).transpose(1, 0, 2).reshape(tc * 128, l1)
    out = np.zeros((b, l1), np.float32)
    out[order] = slots
    return out


_PROG_CACHE = {}


def _get_program(cfg, nkc):
    key = (tuple(sorted(cfg.items())), nkc, ACH, TG, DOTS, GMODE, SINGLE_PACKET)
    if key not in _PROG_CACHE:
        _PROG_CACHE[key] = build_program(cfg, nkc)
    return _PROG_CACHE[key]


def run(trace=False, **inputs):
    from concourse.bass_utils import run_bass_kernel_spmd

    cfg = make_cfg()
    in_maps, nkc, order = host_prep(cfg, **inputs)
    nc = _get_program(cfg, nkc)
    res = run_bass_kernel_spmd(nc, in_maps, list(range(cfg["ncores"])),
                               trace=trace)
    out = host_finish(cfg, [r["outp"] for r in res.results], order)
    return out, res


def kernel(**inputs) -> np.ndarray:
    out, _ = run(trace=False, **inputs)
    return out
